# revision 1
# baseline (speedup 1.0000x reference)
"""Trainium2 Bass kernel for nn_HashingMemory (product-key memory layer).

Data-parallel over tokens: 2048 tokens sharded 256/core across 8 NeuronCores,
keys/query-proj/value-table replicated. Per core:
  1. q = x @ w_q.T + b_q                       (PE, via on-chip transposes)
  2. scores[h,t] = q_ht @ keys_ht.T            (PE)
  3. level-1 top-32 of 512 per (tok, h, t)     (DVE max/max_index/match_replace)
  4. cross-sum top-32 of 1024 per (tok, h)     (DVE) + exact dup correction
  5. rank->subkey-index lookup + softmax       (DVE/ACT)
  6. weighted gather-sum from the 1GiB value
     table: 128 rows x 4KB per token           (SWDGE indirect DMA + PE)

The kernel is self-contained: shapes hardcoded, no file I/O.
"""

import numpy as np
from contextlib import ExitStack

TOK, BLK, NBLK = 256, 128, 2          # tokens per core, per block
H, HALF, NK, KNN, V, D, O = 4, 256, 512, 32, 1024, 1024, 2048
NCORES = 8
SENT = -3.0e38                         # match_replace sentinel / marker value

_CACHE = {}


def _consts_np():
    c = np.zeros((128, 2048), np.float32)
    c[:, :1024] = np.arange(1024, dtype=np.float32)[None, :]
    ltri = np.tril(np.ones((KNN, KNN), np.float32), k=-1)  # ltri[k,j]=1 iff j<k
    c[:, 1024:2048] = ltri.reshape(-1)[None, :]
    return c


def _build_nc():
    import concourse.bass as bass
    import concourse.bacc as bacc
    import concourse.mybir as mybir
    import concourse.tile as tile
    from concourse.masks import make_identity

    F32, U32 = mybir.dt.float32, mybir.dt.uint32
    AX, ALU = mybir.AxisListType, mybir.AluOpType
    AF = mybir.ActivationFunctionType

    nc = bacc.Bacc("TRN2", target_bir_lowering=False, debug=False)
    x_d = nc.dram_tensor("x", [TOK, D], F32, kind="ExternalInput").ap()
    wq_d = nc.dram_tensor("w_q", [O, D], F32, kind="ExternalInput").ap()
    bq_d = nc.dram_tensor("b_q", [O], F32, kind="ExternalInput").ap()
    keys_d = nc.dram_tensor("keys", [H, 2, NK, HALF], F32, kind="ExternalInput").ap()
    vals_d = nc.dram_tensor("values", [NK * NK, V], F32, kind="ExternalInput").ap()
    consts_d = nc.dram_tensor("consts", [128, 2048], F32, kind="ExternalInput").ap()
    out_d = nc.dram_tensor("out", [TOK, V], F32, kind="ExternalOutput").ap()

    with tile.TileContext(nc) as tc, ExitStack() as ctx:
        pc = ctx.enter_context(tc.tile_pool(name="const", bufs=1))
        p_wnat = ctx.enter_context(tc.tile_pool(name="wnat", bufs=2))
        p_knat = ctx.enter_context(tc.tile_pool(name="knat", bufs=8))
        p_sc = ctx.enter_context(tc.tile_pool(name="sc", bufs=1))
        p_sm = ctx.enter_context(tc.tile_pool(name="sm", bufs=2))
        p_big = ctx.enter_context(tc.tile_pool(name="big", bufs=1))
        p_g = ctx.enter_context(tc.tile_pool(name="g", bufs=4))
        p_out = ctx.enter_context(tc.tile_pool(name="outp", bufs=2))
        ps_a = ctx.enter_context(tc.tile_pool(name="psa", bufs=2, space="PSUM"))
        ps_f = ctx.enter_context(tc.tile_pool(name="psf", bufs=2, space="PSUM"))

        # ---------- P0/P1: constants + small loads ----------
        ident = pc.tile([128, 128], F32, tag="ident")
        make_identity(nc, ident[:])
        consts = pc.tile([128, 2048], F32, tag="consts")
        nc.sync.dma_start(consts[:], consts_d[:])
        iota32 = consts[:, 0:KNN]
        iota512 = consts[:, 0:NK]
        iota1024 = consts[:, 0:1024]
        ltri = consts[:, 1024:2048].rearrange("p (k j) -> p k j", k=KNN)

        x_sb = []
        for blk in range(NBLK):
            t = pc.tile([128, D], F32, tag=f"x{blk}")
            nc.sync.dma_start(t[:], x_d[blk * BLK:(blk + 1) * BLK, :])
            x_sb.append(t)

        bq16 = pc.tile([16, 128], F32, tag="bq16")
        nc.sync.dma_start(bq16[:], bq_d.rearrange("(c f) -> c f", c=16))
        psum_bq = ps_a.tile([128, 16], F32, tag="bank", space="PSUM")
        nc.tensor.transpose(out=psum_bq[:], in_=bq16[:], identity=ident[:16, :16])
        bqT = pc.tile([128, 16], F32, tag="bqT")
        nc.vector.tensor_copy(out=bqT[:], in_=psum_bq[:])

        # ---------- P2: x -> x_T [d, tok] ----------
        xT = []  # 8 tiles [128, 256]: d-chunk dc -> tokens
        for dc in range(8):
            t = pc.tile([128, TOK], F32, tag=f"xT{dc}")
            xT.append(t)
        for dc in range(8):
            for blk in range(NBLK):
                pt = ps_a.tile([128, 128], F32, tag="bank", space="PSUM")
                nc.tensor.transpose(
                    out=pt[:], in_=x_sb[blk][:, dc * 128:(dc + 1) * 128],
                    identity=ident[:])
                nc.scalar.activation(
                    out=xT[dc][:, blk * BLK:(blk + 1) * BLK], in_=pt[:],
                    func=AF.Copy)

        # ---------- P3: keys -> keys_T [d, n] per (h, t) ----------
        keysT = {}  # (h, t, dc) -> [128, 512]
        for h in range(H):
            for t2 in range(2):
                for dc in range(2):
                    keysT[h, t2, dc] = pc.tile([128, NK], F32, tag=f"kT{h}{t2}{dc}", name=f"kT{h}{t2}{dc}")
                for n4 in range(4):
                    knat = p_knat.tile([128, HALF], F32, tag="knat")
                    nc.sync.dma_start(
                        knat[:], keys_d[h, t2, n4 * 128:(n4 + 1) * 128, :])
                    for dc in range(2):
                        pt = ps_a.tile([128, 128], F32, tag="bank", space="PSUM")
                        nc.tensor.transpose(
                            out=pt[:], in_=knat[:, dc * 128:(dc + 1) * 128],
                            identity=ident[:])
                        nc.scalar.activation(
                            out=keysT[h, t2, dc][:, n4 * 128:(n4 + 1) * 128],
                            in_=pt[:], func=AF.Copy)

        # ---------- P4: q_T = (x @ w_q.T + b_q).T  [o, tok] ----------
        qT = []  # 16 tiles [128, 256]; chunk c: o in [128c, 128c+128)
        for oc in range(16):
            qT.append(pc.tile([128, TOK], F32, tag=f"qT{oc}", name=f"qT{oc}"))
        for oc in range(16):
            wnat = p_wnat.tile([128, D], F32, tag="wnat")
            nc.sync.dma_start(wnat[:], wq_d[oc * 128:(oc + 1) * 128, :])
            psq = ps_a.tile([128, TOK], F32, tag="bank", space="PSUM")
            for dc in range(8):
                pt = ps_a.tile([128, 128], F32, tag="bank2", space="PSUM")
                nc.tensor.transpose(
                    out=pt[:], in_=wnat[:, dc * 128:(dc + 1) * 128],
                    identity=ident[:])
                wT = p_wnat.tile([128, 128], F32, tag="wT")
                nc.scalar.activation(out=wT[:], in_=pt[:], func=AF.Copy)
                nc.tensor.matmul(out=psq[:], lhsT=wT[:], rhs=xT[dc][:],
                                 start=(dc == 0), stop=(dc == 7))
            # bias-add during PSUM->SBUF copy (bias indexed by o = partition)
            nc.scalar.activation(out=qT[oc][:], in_=psq[:], func=AF.Identity,
                                 bias=bqT[:, oc:oc + 1], scale=1.0)

        # ---------- per-block compute ----------
        sc = {}     # (blk, h, t) -> scores tile [128, 512] (mutated by topk)
        sv = {}     # (blk, h, t) -> top-32 values  [128, 32] f32 desc
        ivf = {}    # (blk, h, t) -> top-32 subkey indices as f32 [128, 32]
        idxT = {}   # blk -> [128 (h,k), 128 tok] u32
        wT_f = {}   # blk -> [128 (h,k), 128 tok] f32

        def stage_scores(blk):
            for h in range(H):
                for t2 in range(2):
                    pss = ps_a.tile([128, NK], F32, tag="bank", space="PSUM")
                    for dc in range(2):
                        oc = h * 4 + t2 * 2 + dc
                        nc.tensor.matmul(
                            out=pss[:],
                            lhsT=qT[oc][:, blk * BLK:(blk + 1) * BLK],
                            rhs=keysT[h, t2, dc][:],
                            start=(dc == 0), stop=(dc == 1))
                    t = p_sc.tile([128, NK], F32, tag=f"sc{h}{t2}")
                    nc.vector.tensor_copy(out=t[:], in_=pss[:])
                    sc[blk, h, t2] = t

        def topk_rounds(cur, vals, posf, iota, n):
            """4x (max, max_index, match_replace) + exact dup correction.
            cur [128, n] mutated; vals [128,32] f32; posf [128,32] f32."""
            pos_u = p_sm.tile([128, 8], mybir.dt.uint32, tag="posu8")
            for r in range(4):
                s8 = vals[:, r * 8:(r + 1) * 8]
                nc.vector.max(out=s8, in_=cur[:])
                nc.vector.max_index(out=pos_u[:], in_max=s8, in_values=cur[:])
                nc.vector.tensor_copy(out=posf[:, r * 8:(r + 1) * 8], in_=pos_u[:])
                nc.vector.match_replace(out=cur[:], in_to_replace=s8,
                                        in_values=cur[:], imm_value=SENT)
            # markers: positions actually replaced (exact, handles dups)
            marker = p_big.tile([128, 1024], F32, tag="marker")
            nc.vector.tensor_scalar(out=marker[:, :n], in0=cur[:], scalar1=SENT,
                                    scalar2=None, op0=ALU.is_equal)
            summark = p_sm.tile([128, 1], F32, tag="summark")
            junk = p_big.tile([128, 1024], F32, tag="junk")
            nc.vector.scalar_tensor_tensor(
                out=junk[:, :n], in0=marker[:, :n], scalar=1.0, in1=iota,
                op0=ALU.mult, op1=ALU.mult, accum_out=summark[:])
            sumpos = p_sm.tile([128, 1], F32, tag="sumpos")
            nc.vector.tensor_reduce(out=sumpos[:], in_=posf[:], axis=AX.X,
                                    op=ALU.add)
            diff = p_sm.tile([128, 1], F32, tag="diff")
            nc.vector.tensor_tensor(out=diff[:], in0=summark[:], in1=sumpos[:],
                                    op=ALU.subtract)
            # dup[k] = sum_{j<k} (pos[k]==pos[j]); corrected pos += dup*diff
            eqm = p_big.tile([128, KNN, KNN], F32, tag="eqm")
            nc.vector.tensor_tensor(
                out=eqm[:], in0=posf[:].to_broadcast([128, KNN, KNN]),
                in1=posf[:].unsqueeze(1).broadcast_to([128, KNN, KNN]),
                op=ALU.is_equal)
            nc.vector.tensor_tensor(out=eqm[:], in0=eqm[:], in1=ltri,
                                    op=ALU.mult)
            cnt = p_sm.tile([128, KNN], F32, tag="cnt")
            nc.vector.tensor_reduce(out=cnt[:], in_=eqm[:], axis=AX.X, op=ALU.add)
            nc.vector.tensor_scalar(out=cnt[:], in0=cnt[:], scalar1=diff[:, :1],
                                    scalar2=None, op0=ALU.mult)
            nc.vector.tensor_tensor(out=posf[:], in0=posf[:], in1=cnt[:],
                                    op=ALU.add)

        def stage_topk1(blk):
            for h in range(H):
                for t2 in range(2):
                    v = p_sm.tile([128, KNN], F32, tag=f"sv{h}{t2}")
                    pf = p_sm.tile([128, KNN], F32, tag=f"ivf{h}{t2}")
                    topk_rounds(sc[blk, h, t2], v, pf, iota512, NK)
                    sv[blk, h, t2] = v
                    ivf[blk, h, t2] = pf

        def stage_cross(blk):
            idxf = p_sm.tile([128, 128], F32, tag="idxf")
            wf = p_sm.tile([128, 128], F32, tag="wf")
            for h in range(H):
                s1, s2 = sv[blk, h, 0], sv[blk, h, 1]
                i1f, i2f = ivf[blk, h, 0], ivf[blk, h, 1]
                i1s = p_sm.tile([128, KNN], F32, tag="i1s")
                nc.vector.tensor_scalar_mul(i1s[:], i1f[:], float(NK))
                all_s = p_big.tile([128, KNN, KNN], F32, tag="alls")
                nc.vector.tensor_tensor(
                    out=all_s[:], in0=s1[:].to_broadcast([128, KNN, KNN]),
                    in1=s2[:].unsqueeze(1).broadcast_to([128, KNN, KNN]),
                    op=ALU.add)
                all_i = p_big.tile([128, KNN, KNN], F32, tag="alli")
                nc.vector.tensor_tensor(
                    out=all_i[:], in0=i1s[:].to_broadcast([128, KNN, KNN]),
                    in1=i2f[:].unsqueeze(1).broadcast_to([128, KNN, KNN]),
                    op=ALU.add)
                bs_v = p_sm.tile([128, KNN], F32, tag="bsv")
                posf = p_sm.tile([128, KNN], F32, tag="posf")
                all_s_flat = all_s[:].rearrange("p a b -> p (a b)")
                topk_rounds(all_s_flat, bs_v, posf, iota1024, 1024)
                # pos -> (a, b) ranks
                pos_u = p_sm.tile([128, KNN], mybir.dt.uint32, tag="poscu")
                nc.vector.tensor_copy(out=pos_u[:], in_=posf[:])
                a_u = p_sm.tile([128, KNN], mybir.dt.uint32, tag="au")
                b_u = p_sm.tile([128, KNN], mybir.dt.uint32, tag="bu")
                nc.vector.tensor_scalar(out=a_u[:], in0=pos_u[:], scalar1=5,
                                        scalar2=None,
                                        op0=ALU.logical_shift_right)
                nc.vector.tensor_scalar(out=b_u[:], in0=pos_u[:], scalar1=31,
                                        scalar2=None, op0=ALU.bitwise_and)
                af = p_sm.tile([128, KNN], F32, tag="af")
                bf = p_sm.tile([128, KNN], F32, tag="bf")
                nc.vector.tensor_copy(out=af[:], in_=a_u[:])
                nc.vector.tensor_copy(out=bf[:], in_=b_u[:])
                # rank lookup: g1 = i1s[a], g2 = i2f[b] via eq + reduce
                g1 = p_sm.tile([128, KNN], F32, tag="g1")
                g2 = p_sm.tile([128, KNN], F32, tag="g2")
                eqm = p_big.tile([128, KNN, KNN], F32, tag="eqm")
                nc.vector.tensor_tensor(
                    out=eqm[:], in0=af[:].to_broadcast([128, KNN, KNN]),
                    in1=iota32.unsqueeze(1).broadcast_to([128, KNN, KNN]),
                    op=ALU.is_equal)
                nc.vector.tensor_tensor(
                    out=eqm[:], in0=eqm[:],
                    in1=i1s[:].unsqueeze(1).broadcast_to([128, KNN, KNN]),
                    op=ALU.mult)
                nc.vector.tensor_reduce(out=g1[:], in_=eqm[:], axis=AX.X,
                                        op=ALU.add)
                eqm2 = p_big.tile([128, KNN, KNN], F32, tag="eqm")
                nc.vector.tensor_tensor(
                    out=eqm2[:], in0=bf[:].to_broadcast([128, KNN, KNN]),
                    in1=iota32.unsqueeze(1).broadcast_to([128, KNN, KNN]),
                    op=ALU.is_equal)
                nc.vector.tensor_tensor(
                    out=eqm2[:], in0=eqm2[:],
                    in1=i2f[:].unsqueeze(1).broadcast_to([128, KNN, KNN]),
                    op=ALU.mult)
                nc.vector.tensor_reduce(out=g2[:], in_=eqm2[:], axis=AX.X,
                                        op=ALU.add)
                nc.vector.tensor_tensor(out=idxf[:, h * KNN:(h + 1) * KNN],
                                        in0=g1[:], in1=g2[:], op=ALU.add)
                # softmax over the 32 (bs_v is descending: max = col 0)
                negm = p_sm.tile([128, 1], F32, tag="negm")
                nc.vector.tensor_scalar_mul(negm[:], bs_v[:, 0:1], -1.0)
                e = p_sm.tile([128, KNN], F32, tag="esm")
                nc.scalar.activation(out=e[:], in_=bs_v[:], func=AF.Exp,
                                     bias=negm[:, 0:1], scale=1.0)
                ssum = p_sm.tile([128, 1], F32, tag="ssum")
                nc.vector.tensor_reduce(out=ssum[:], in_=e[:], axis=AX.X,
                                        op=ALU.add)
                rec = p_sm.tile([128, 1], F32, tag="rec")
                nc.vector.reciprocal(rec[:], ssum[:])
                nc.vector.tensor_scalar_mul(wf[:, h * KNN:(h + 1) * KNN], e[:],
                                            rec[:, 0:1])
            # transpose -> [(h,k), tok]
            pt_i = ps_a.tile([128, 128], F32, tag="bank", space="PSUM")
            nc.tensor.transpose(out=pt_i[:], in_=idxf[:], identity=ident[:])
            it = p_sm.tile([128, 128], mybir.dt.uint32, tag="idxT")
            nc.vector.tensor_copy(out=it[:], in_=pt_i[:])
            idxT[blk] = it
            pt_w = ps_a.tile([128, 128], F32, tag="bank", space="PSUM")
            nc.tensor.transpose(out=pt_w[:], in_=wf[:], identity=ident[:])
            wt = p_sm.tile([128, 128], F32, tag="wT")
            nc.scalar.activation(out=wt[:], in_=pt_w[:], func=AF.Copy)
            wT_f[blk] = wt

        def stage_gather(blk):
            # out_T accumulation: psum column t of chunk c = (V-chunk c of
            # token t's weighted sum). Two banks hold 4 chunks x 128 tokens
            # each; all engine APs start at partition 0 (HW requirement).
            outt = p_out.tile([128, V], F32, tag="OUT")
            it, wt = idxT[blk], wT_f[blk]
            pcc = [ps_f.tile([128, NK], F32, tag=f"fc{i}", space="PSUM",
                             name=f"fc{i}") for i in range(2)]
            for t in range(BLK):
                G = p_g.tile([128, V], F32, tag="G")
                nc.gpsimd.indirect_dma_start(
                    out=G[:], out_offset=None, in_=vals_d[:],
                    in_offset=bass.IndirectOffsetOnAxis(
                        ap=it[:, t:t + 1], axis=0))
                for c in range(8):
                    cc, ci = divmod(c, 4)
                    col = ci * 128 + t
                    nc.tensor.matmul(
                        out=pcc[cc][:, col:col + 1],
                        lhsT=G[:, c * 128:(c + 1) * 128], rhs=wt[:, t:t + 1],
                        start=True, stop=True)
            for cc in range(2):
                oT = p_out.tile([128, NK], F32, tag="oT")
                nc.vector.tensor_copy(out=oT[:], in_=pcc[cc][:])
                for ci in range(4):
                    c = cc * 4 + ci
                    pt = ps_a.tile([128, 128], F32, tag="bank", space="PSUM")
                    nc.tensor.transpose(
                        out=pt[:], in_=oT[:, ci * 128:(ci + 1) * 128],
                        identity=ident[:])
                    nc.scalar.activation(
                        out=outt[:, c * 128:(c + 1) * 128], in_=pt[:],
                        func=AF.Copy)
            nc.sync.dma_start(out_d[blk * BLK:(blk + 1) * BLK, :], outt[:])

        # emission order chosen so block-1 top-k (DVE) overlaps block-0
        # gathers (DMA/PE)
        stage_scores(0)
        stage_topk1(0)
        stage_cross(0)
        stage_scores(1)
        stage_gather(0)
        stage_topk1(1)
        stage_cross(1)
        stage_gather(1)

    nc.compile()
    return nc


def _get_nc():
    if "nc" not in _CACHE:
        _CACHE["nc"] = _build_nc()
    return _CACHE["nc"]


def kernel(**inputs):
    x = np.ascontiguousarray(np.asarray(inputs["x"], dtype=np.float32))
    w_q = np.ascontiguousarray(np.asarray(inputs["w_q"], dtype=np.float32))
    b_q = np.ascontiguousarray(np.asarray(inputs["b_q"], dtype=np.float32))
    keys = np.ascontiguousarray(np.asarray(inputs["keys"], dtype=np.float32))
    values = np.ascontiguousarray(np.asarray(inputs["values"], dtype=np.float32))
    B, S, Din = x.shape
    xf = x.reshape(B * S, Din)
    consts = _consts_np()

    from concourse.bass_utils import run_bass_kernel_spmd

    nc = _get_nc()
    in_maps = []
    for c in range(NCORES):
        in_maps.append({
            "x": np.ascontiguousarray(xf[c * TOK:(c + 1) * TOK]),
            "w_q": w_q, "b_q": b_q, "keys": keys, "values": values,
            "consts": consts,
        })
    res = run_bass_kernel_spmd(nc, in_maps, core_ids=list(range(NCORES)),
                               **_CACHE.get("run_kwargs", {}))
    _CACHE["last_result"] = res
    out = np.concatenate([r["out"] for r in res.results], axis=0)
    return out.reshape(B, S, V).astype(np.float32)


if __name__ == "__main__":
    rng = np.random.default_rng(0)
    ins = {
        "x": rng.standard_normal((1, 2048, 1024), dtype=np.float32),
        "w_q": rng.standard_normal((2048, 1024), dtype=np.float32) * 0.03,
        "b_q": rng.standard_normal((2048,), dtype=np.float32) * 0.01,
        "keys": (rng.random((4, 2, 512, 256), dtype=np.float32) - 0.5) / 8,
        "values": rng.standard_normal((262144, 1024), dtype=np.float32) * 0.03,
    }
    out = kernel(**ins)
    print(out.shape, out.dtype, np.abs(out).max())

